# revision 34
# baseline (speedup 1.0000x reference)
"""Multi-head attention (B=4, N=2048, E=768, H=8) on 8 TRN2 NeuronCores.

Sharding: core c handles batch b = c//2 and head-group g = c%2 (4 heads).
Column-parallel Wqkv, row-parallel Wproj; each core emits a partial
projection [N, E]; host sums the two partials per batch and adds bproj.

Per-core device pipeline (all matmuls fp32r = full PE rate):
  1. QKV projections from host-pre-transposed xT [E, N]:
       qT/kT in [d, n] layout (head-dim on partitions) for the energy
       matmul; V in natural [n, d] layout, augmented with a ones column
       so the attention-V matmul also produces softmax denominators.
  2. Per head: flash-style attention per 512-wide query block: energy into
     PSUM, exp on ScalarE (no max subtraction -- energies are O(20)),
     unnormalized att @ V_aug accumulated on PE -> oT [97, N]
     (row 96 = softmax denominators).
  3. Per (head, block): normalization deferred to oT: denominator row
     scattered across partitions (small gpsimd DMA), DVE reciprocal,
     gathered back, broadcast across partitions with a K=1 ones-matmul,
     one elementwise multiply on oT[0:96].
  4. Output projection accumulates all 4 heads in PSUM. The post-softmax
     SCALING of the reference is folded into Wproj on the host.
"""
import sys

sys.path.insert(0, "/opt/trn_rl_repo")

import numpy as np

N_CORES = 8
B, N, E, H = 4, 2048, 768, 8
D = E // H  # 96
HPC = H // 2  # heads per core: 4
SCALING = float(D) ** -0.5
DA = D + 1  # 97: head dims + ones column
NB = N // 512  # 4 query blocks of 512
NT = N // 128  # 16 key tiles of 128
KC = E // 128  # 6 contraction chunks for the projections

_compiled = None


def _build(scat_sync=False, eps_bufs=2, x_blocks="k6", norm_inplace=False):
    import concourse.bass as bass
    import concourse.tile as tile
    import concourse.mybir as mybir
    from concourse import bacc
    from contextlib import ExitStack

    f32 = mybir.dt.float32
    f32r = mybir.dt.float32r

    nc = bacc.Bacc("TRN2", target_bir_lowering=False, debug=False,
                   num_devices=N_CORES)

    xT = nc.dram_tensor("xT", [E, N], f32, kind="ExternalInput").ap()
    wq = nc.dram_tensor("wq", [E, HPC, D], f32, kind="ExternalInput").ap()
    wk = nc.dram_tensor("wk", [E, HPC, D], f32, kind="ExternalInput").ap()
    wv = nc.dram_tensor("wv", [E, HPC, DA], f32, kind="ExternalInput").ap()
    bq = nc.dram_tensor("bq", [HPC, D], f32, kind="ExternalInput").ap()
    bk = nc.dram_tensor("bk", [HPC, D], f32, kind="ExternalInput").ap()
    bv = nc.dram_tensor("bv", [HPC, DA], f32, kind="ExternalInput").ap()
    wp = nc.dram_tensor("wp", [HPC, D, E], f32, kind="ExternalInput").ap()
    ones = nc.dram_tensor("ones", [1, 128], f32, kind="ExternalInput").ap()
    y = nc.dram_tensor("y", [N, E], f32, kind="ExternalOutput").ap()

    with tile.TileContext(nc, num_cores=N_CORES) as tc, ExitStack() as top:
        # Long-lived SBUF tiles (~102.5 KB/partition)
        persist = top.enter_context(tc.tile_pool(name="persist", bufs=1))
        qT = persist.tile([128, HPC, N], f32r)       # [d, h, n] rows 0:96
        kT = persist.tile([128, HPC, N], f32r)
        vA = persist.tile([128, NT, HPC * DA], f32r)  # [n%128, ntile, h*97+da]
        wp_sb = persist.tile([128, HPC, E], f32r)    # [d, h, e] rows 0:96
        ones_sb = persist.tile([1, 128], f32r)
        ones96_sb = persist.tile([DA, 128], f32r)
        bq_sb = persist.tile([128, HPC], f32)        # [d, h]
        bk_sb = persist.tile([128, HPC], f32)
        bvb_sb = persist.tile([128, HPC * DA], f32)  # bv broadcast all parts

        nc.gpsimd.dma_start(out=wp_sb[0:D, :, :],
                            in_=wp.rearrange("h d e -> d h e"))
        nc.gpsimd.dma_start(out=ones_sb, in_=ones)
        nc.gpsimd.dma_start(out=ones96_sb[D:DA, :], in_=ones)
        nc.sync.dma_start(out=bq_sb[0:D, :], in_=bq.rearrange("h d -> d h"))
        # dummy exp to pull the ACT exp-table load into phase 1
        warm_sb = persist.tile([128, HPC], f32)
        nc.scalar.activation(out=warm_sb[0:D, :], in_=bq_sb[0:D, :],
                             func=mybir.ActivationFunctionType.Exp)
        nc.sync.dma_start(out=bk_sb[0:D, :], in_=bk.rearrange("h d -> d h"))
        bv_flat = bv.rearrange("h d -> (h d)")
        nc.sync.dma_start(out=bvb_sb,
                          in_=bass.AP(tensor=bv_flat.tensor,
                                      offset=bv_flat.offset,
                                      ap=[[0, 128], *bv_flat.ap]))

        # ---------------- Phase 1: QKV projections ----------------
        with ExitStack() as p1:
            pool1 = p1.enter_context(tc.tile_pool(name="p1sb", bufs=1))
            xT_sb = pool1.tile([128, KC, N], f32r)   # [e%128, echunk, n]
            wq_sb = pool1.tile([128, KC, HPC, D], f32r)
            wk_sb = pool1.tile([128, KC, HPC, D], f32r)
            wv_sb = pool1.tile([128, KC, HPC, DA], f32r)
            # x split along contraction chunks (k3 default): keeps 8KB
            # DMA lines and lets the first QKV matmuls start early.
            xTr = xT.rearrange("(k p) n -> p k n", p=128)
            nc.gpsimd.dma_start(
                out=wv_sb, in_=wv.rearrange("(k p) h d -> p k h d", p=128))
            if x_blocks == "sync":
                # x rides the HWDGE queues (disjoint from the gpsimd/SWDGE
                # pool carrying the weights) as plain fp32; the fp32r
                # rounding happens as an in-place DVE copy per chunk.
                for kb in range(KC):
                    nc.sync.dma_start(out=xT_sb[:, kb].bitcast(f32),
                                      in_=xTr[:, kb])
                    nc.vector.tensor_copy(out=xT_sb[:, kb],
                                          in_=xT_sb[:, kb].bitcast(f32))
            elif x_blocks in ("k3", "k6"):
                # split along the contraction chunks: keeps 8KB DMA lines
                # and lets the first QKV matmuls start early in the load
                step = 2 if x_blocks == "k3" else 1
                for kb in range(0, KC, step):
                    nc.gpsimd.dma_start(out=xT_sb[:, kb:kb + step],
                                        in_=xTr[:, kb:kb + step])
            elif x_blocks:
                w = N // x_blocks
                for jb in range(x_blocks):
                    jn = slice(jb * w, (jb + 1) * w)
                    nc.gpsimd.dma_start(out=xT_sb[:, :, jn], in_=xTr[:, :, jn])
            else:
                nc.gpsimd.dma_start(out=xT_sb, in_=xTr)
            nc.gpsimd.dma_start(
                out=wk_sb, in_=wk.rearrange("(k p) h d -> p k h d", p=128))
            nc.gpsimd.dma_start(
                out=wq_sb, in_=wq.rearrange("(k p) h d -> p k h d", p=128))

            qk_ps = p1.enter_context(
                tc.tile_pool(name="qk_ps", bufs=3, space="PSUM"))
            v_ps = p1.enter_context(
                tc.tile_pool(name="v_ps", bufs=2, space="PSUM"))

            # V (+ones col): natural [n, h*97+d] = xT[k][:, ntile].T @ Wv[k]
            for t in range(NT):
                ps = v_ps.tile([128, HPC * DA], f32, tag="v")
                for k in range(KC):
                    nc.tensor.matmul(
                        ps, xT_sb[:, k, t * 128:(t + 1) * 128],
                        wv_sb[:, k, :, :].rearrange("p h d -> p (h d)"),
                        start=(k == 0), stop=(k == KC - 1))
                nc.vector.tensor_tensor(
                    out=vA[:, t, :], in0=ps, in1=bvb_sb,
                    op=mybir.AluOpType.add)

            # qT/kT: [d, n] = sum_k Wq[k][:, h-cols].T @ xT[k]
            # head-outer so head h's attention can start while h+1 projects
            for h in range(HPC):
                for (w_sb, b_sb, dst) in ((wk_sb, bk_sb, kT),
                                          (wq_sb, bq_sb, qT)):
                    for j in range(NB):
                        ps = qk_ps.tile([D, 512], f32, tag="qk")
                        for k in range(KC):
                            nc.tensor.matmul(
                                ps, w_sb[:, k, h, :],
                                xT_sb[:, k, j * 512:(j + 1) * 512],
                                start=(k == 0), stop=(k == KC - 1))
                        nc.vector.tensor_scalar(
                            out=dst[0:D, h, j * 512:(j + 1) * 512],
                            in0=ps, scalar1=b_sb[0:D, h:h + 1],
                            scalar2=None, op0=mybir.AluOpType.add)

        # ------- Phases 2-5: attention + normalize + projection -------
        # Query-block-outer so the projection of block J overlaps the
        # attention of block J+1.
        with ExitStack() as p2:
            oT = p2.enter_context(tc.tile_pool(name="p2sb", bufs=1)).tile(
                [128, HPC, N], f32r)             # rows 0:96 o, 96 denom
            expool = p2.enter_context(tc.tile_pool(name="p2ex", bufs=1))
            small2 = p2.enter_context(tc.tile_pool(name="p2small", bufs=3))
            pool5 = p2.enter_context(tc.tile_pool(name="p5sb", bufs=3))
            e_ps = p2.enter_context(
                tc.tile_pool(name="e_ps", bufs=eps_bufs, space="PSUM"))
            o_ps = p2.enter_context(
                tc.tile_pool(name="o_ps", bufs=1, space="PSUM"))
            bc_ps = p2.enter_context(
                tc.tile_pool(name="bc_ps", bufs=1, space="PSUM"))
            p_ps = p2.enter_context(
                tc.tile_pool(name="p_ps", bufs=1, space="PSUM"))
            for j in range(NB):
                jq = slice(j * 512, (j + 1) * 512)
                for h in range(HPC):
                    ex = expool.tile([128, NT, 512], f32r, tag="ex")
                    ops = o_ps.tile([DA, 512], f32, tag="o")
                    gsz = 4 // eps_bufs
                    for grp in range(NT // gsz):
                        eps = e_ps.tile([128, gsz, 512], f32, tag="e")
                        for i in range(gsz):
                            t = grp * gsz + i
                            nc.tensor.matmul(
                                eps[:, i, :], kT[0:D, h, t * 128:(t + 1) * 128],
                                qT[0:D, h, jq], start=True, stop=True)
                        nc.scalar.activation(
                            out=ex[:, grp * gsz:(grp + 1) * gsz, :], in_=eps,
                            func=mybir.ActivationFunctionType.Exp)
                    for t in range(NT):
                        nc.tensor.matmul(
                            ops, vA[:, t, h * DA:(h + 1) * DA], ex[:, t, :],
                            start=(t == 0), stop=(t == NT - 1))
                    nc.vector.tensor_copy(out=oT[0:DA, h, jq], in_=ops)

                    # normalize (h, j): recip of denom row, broadcast, mult
                    # (borrows an energy-psum slot; first bank of the pair)
                    bc = bc_ps.tile([128, 512], f32, tag="bc")
                    with nc.allow_low_precision(
                            reason="f32r is 4-byte; recip is full fp32"):
                        if norm_inplace:
                            nc.vector.reciprocal(out=oT[D:DA, h, jq],
                                                 in_=ops[D:DA, :])
                            nc.tensor.matmul(bc, ones96_sb[D:DA, :],
                                             oT[D:DA, h, jq],
                                             start=True, stop=True,
                                             tile_position=(D - D % 32, 0))
                        else:
                            scat = small2.tile([128, NB], f32r, tag="scat")
                            recip = small2.tile([1, 512], f32r, tag="recip")
                            dma_eng = nc.sync if scat_sync else nc.gpsimd
                            dma_eng.dma_start(out=scat, in_=oT[D:DA, h, jq])
                            nc.vector.reciprocal(out=scat, in_=scat)
                            dma_eng.dma_start(out=recip, in_=scat)
                            nc.tensor.matmul(bc, ones_sb, recip,
                                             start=True, stop=True)
                    nc.vector.tensor_tensor(
                        out=oT[0:D, h, jq], in0=oT[0:D, h, jq],
                        in1=bc[0:D, :], op=mybir.AluOpType.mult)

                # projection of this query block (4 n-tiles of 128)
                for t in range(4 * j, 4 * (j + 1)):
                    tn = slice(t * 128, (t + 1) * 128)
                    ot = pool5.tile([128, E], f32, tag="out")
                    for (lo, hi) in ((0, 512), (512, E)):
                        ps = p_ps.tile([128, hi - lo], f32, tag=f"p{lo}")
                        for h in range(HPC):
                            nc.tensor.matmul(ps, oT[0:D, h, tn],
                                             wp_sb[0:D, h, lo:hi],
                                             start=(h == 0),
                                             stop=(h == HPC - 1))
                        nc.vector.tensor_copy(out=ot[:, lo:hi], in_=ps)
                    nc.sync.dma_start(out=y[tn, :], in_=ot)

    nc.compile()
    return nc


def _get_compiled():
    global _compiled
    if _compiled is None:
        _compiled = _build()
    return _compiled


def make_in_maps(x, Wqkv, bqkv, Wproj, bproj):
    """Host-side sharding: per-core input dict."""
    x = np.asarray(x, dtype=np.float32)
    Wqkv = np.asarray(Wqkv, dtype=np.float32)
    bqkv = np.asarray(bqkv, dtype=np.float32)
    Wproj = np.asarray(Wproj, dtype=np.float32)

    Wr = Wqkv.reshape(E, H, D, 3)
    br = bqkv.reshape(H, D, 3)
    ones = np.ones((1, 128), dtype=np.float32)
    in_maps = []
    for c in range(N_CORES):
        b, g = divmod(c, 2)
        hs = slice(g * HPC, (g + 1) * HPC)
        wv_aug = np.zeros((E, HPC, DA), dtype=np.float32)
        wv_aug[:, :, :D] = Wr[:, hs, :, 2]
        bv_aug = np.zeros((HPC, DA), dtype=np.float32)
        bv_aug[:, :D] = br[hs, :, 2]
        bv_aug[:, D] = 1.0
        in_maps.append({
            "xT": np.ascontiguousarray(x[b].T),
            "wq": np.ascontiguousarray(Wr[:, hs, :, 0]),
            "wk": np.ascontiguousarray(Wr[:, hs, :, 1]),
            "wv": wv_aug,
            "bq": np.ascontiguousarray(br[hs, :, 0]),
            "bk": np.ascontiguousarray(br[hs, :, 1]),
            "bv": bv_aug,
            "wp": np.ascontiguousarray(
                (SCALING * Wproj[g * HPC * D:(g + 1) * HPC * D, :])
                .reshape(HPC, D, E)),
            "ones": ones,
        })
    return in_maps


def combine_outputs(results, bproj):
    bproj = np.asarray(bproj, dtype=np.float32)
    out = np.empty((B, N, E), dtype=np.float32)
    for b in range(B):
        out[b] = results[2 * b]["y"] + results[2 * b + 1]["y"] + bproj
    return out


def kernel(x, Wqkv, bqkv, Wproj, bproj):
    import time
    from concourse.bass_utils import run_bass_kernel_spmd
    nc = _get_compiled()
    in_maps = make_in_maps(x, Wqkv, bqkv, Wproj, bproj)
    last_exc = None
    for attempt in range(3):
        try:
            res = run_bass_kernel_spmd(nc, in_maps,
                                       core_ids=list(range(N_CORES)),
                                       trace=False)
            return combine_outputs(res.results, bproj)
        except Exception as e:  # transient device/terminal failures
            last_exc = e
            time.sleep(10.0)
    raise last_exc


# revision 37
# speedup vs baseline: 1.4879x; 1.4879x over previous
"""Multi-head attention (B=4, N=2048, E=768, H=8) on 8 TRN2 NeuronCores.

Sharding: core c handles batch b = c//2 and head-group g = c%2 (4 heads).
Column-parallel Wqkv, row-parallel Wproj; each core emits a partial
projection [N, E]; host sums the two partials per batch and adds bproj.

Per-core device pipeline (all matmuls fp32r = full PE rate):
  1. QKV projections from host-pre-transposed xT [E, N]:
       qT/kT in [d, n] layout (head-dim on partitions) for the energy
       matmul; V in natural [n, d] layout, augmented with a ones column
       so the attention-V matmul also produces softmax denominators.
  2. Per head: flash-style attention per 512-wide query block: energy into
     PSUM, exp on ScalarE (no max subtraction -- energies are O(20)),
     unnormalized att @ V_aug accumulated on PE -> oT [97, N]
     (row 96 = softmax denominators).
  3. Per (head, block): normalization deferred to oT: denominator row
     scattered across partitions (small gpsimd DMA), DVE reciprocal,
     gathered back, broadcast across partitions with a K=1 ones-matmul,
     one elementwise multiply on oT[0:96].
  4. Output projection accumulates all 4 heads in PSUM. The post-softmax
     SCALING of the reference is folded into Wproj on the host.
"""
import sys

sys.path.insert(0, "/opt/trn_rl_repo")

import numpy as np

N_CORES = 8
B, N, E, H = 4, 2048, 768, 8
D = E // H  # 96
HPC = H // 2  # heads per core: 4
SCALING = float(D) ** -0.5
DA = D + 1  # 97: head dims + ones column
NB = N // 512  # 4 query blocks of 512
NT = N // 128  # 16 key tiles of 128
KC = E // 128  # 6 contraction chunks for the projections

_compiled = None


def _build(scat_sync=False, eps_bufs=2, x_blocks="k6", norm_inplace=False):
    import concourse.bass as bass
    import concourse.tile as tile
    import concourse.mybir as mybir
    from concourse import bacc
    from contextlib import ExitStack

    f32 = mybir.dt.float32
    f32r = mybir.dt.float32r

    nc = bacc.Bacc("TRN2", target_bir_lowering=False, debug=False,
                   num_devices=N_CORES)

    xT = nc.dram_tensor("xT", [E, N], f32, kind="ExternalInput").ap()
    wq = nc.dram_tensor("wq", [E, HPC, D], f32, kind="ExternalInput").ap()
    wk = nc.dram_tensor("wk", [E, HPC, D], f32, kind="ExternalInput").ap()
    wv = nc.dram_tensor("wv", [E, HPC, DA], f32, kind="ExternalInput").ap()
    bq = nc.dram_tensor("bq", [HPC, D], f32, kind="ExternalInput").ap()
    bk = nc.dram_tensor("bk", [HPC, D], f32, kind="ExternalInput").ap()
    bv = nc.dram_tensor("bv", [HPC, DA], f32, kind="ExternalInput").ap()
    wp = nc.dram_tensor("wp", [HPC, D, E], f32, kind="ExternalInput").ap()
    ones = nc.dram_tensor("ones", [1, 128], f32, kind="ExternalInput").ap()
    y = nc.dram_tensor("y", [N, E], f32, kind="ExternalOutput").ap()

    with tile.TileContext(nc, num_cores=N_CORES) as tc, ExitStack() as top:
        # Long-lived SBUF tiles (~102.5 KB/partition)
        persist = top.enter_context(tc.tile_pool(name="persist", bufs=1))
        qT = persist.tile([128, HPC, N], f32r)       # [d, h, n] rows 0:96
        kT = persist.tile([128, HPC, N], f32r)
        vA = persist.tile([128, NT, HPC * DA], f32r)  # [n%128, ntile, h*97+da]
        wp_sb = persist.tile([128, HPC, E], f32r)    # [d, h, e] rows 0:96
        ones_sb = persist.tile([1, 128], f32r)
        ones96_sb = persist.tile([DA, 128], f32r)
        bq_sb = persist.tile([128, HPC], f32)        # [d, h]
        bk_sb = persist.tile([128, HPC], f32)
        bvb_sb = persist.tile([128, HPC * DA], f32)  # bv broadcast all parts

        # PE warmup: dummy matmuls on a memset tile while the input DMAs
        # are in flight -- fills the startup gap and un-throttles the HAM
        # clock gate before the first real matmul.
        warm_mm = persist.tile([128, 512], f32r)
        nc.vector.memset(warm_mm.bitcast(f32), 1.0)
        nc.vector.tensor_copy(out=warm_mm, in_=warm_mm.bitcast(f32))
        with ExitStack() as wps_ctx:
            w_ps = wps_ctx.enter_context(
                tc.tile_pool(name="w_ps", bufs=1, space="PSUM"))
            wp0 = w_ps.tile([128, 512], f32, tag="w")
            for i in range(24):
                nc.tensor.matmul(wp0, warm_mm[:, 0:128], warm_mm,
                                 start=(i == 0), stop=(i == 23))
        nc.gpsimd.dma_start(out=wp_sb[0:D, :, :],
                            in_=wp.rearrange("h d e -> d h e"))
        nc.gpsimd.dma_start(out=ones_sb, in_=ones)
        nc.gpsimd.dma_start(out=ones96_sb[D:DA, :], in_=ones)
        nc.sync.dma_start(out=bq_sb[0:D, :], in_=bq.rearrange("h d -> d h"))
        # dummy exp to pull the ACT exp-table load into phase 1
        warm_sb = persist.tile([128, HPC], f32)
        nc.scalar.activation(out=warm_sb[0:D, :], in_=bq_sb[0:D, :],
                             func=mybir.ActivationFunctionType.Exp)
        nc.sync.dma_start(out=bk_sb[0:D, :], in_=bk.rearrange("h d -> d h"))
        bv_flat = bv.rearrange("h d -> (h d)")
        nc.sync.dma_start(out=bvb_sb,
                          in_=bass.AP(tensor=bv_flat.tensor,
                                      offset=bv_flat.offset,
                                      ap=[[0, 128], *bv_flat.ap]))

        # ---------------- Phase 1: QKV projections ----------------
        with ExitStack() as p1:
            pool1 = p1.enter_context(tc.tile_pool(name="p1sb", bufs=1))
            xT_sb = pool1.tile([128, KC, N], f32r)   # [e%128, echunk, n]
            wq_sb = pool1.tile([128, KC, HPC, D], f32r)
            wk_sb = pool1.tile([128, KC, HPC, D], f32r)
            wv_sb = pool1.tile([128, KC, HPC, DA], f32r)
            # x split along contraction chunks (k3 default): keeps 8KB
            # DMA lines and lets the first QKV matmuls start early.
            xTr = xT.rearrange("(k p) n -> p k n", p=128)
            wvr = wv.rearrange("(k p) h d -> p k h d", p=128)
            for kb in range(0, KC, 2):
                nc.gpsimd.dma_start(out=wv_sb[:, kb:kb + 2],
                                    in_=wvr[:, kb:kb + 2])
            if x_blocks == "sync":
                # x rides the HWDGE queues (disjoint from the gpsimd/SWDGE
                # pool carrying the weights) as plain fp32; the fp32r
                # rounding happens as an in-place DVE copy per chunk.
                for kb in range(KC):
                    nc.sync.dma_start(out=xT_sb[:, kb].bitcast(f32),
                                      in_=xTr[:, kb])
                    nc.vector.tensor_copy(out=xT_sb[:, kb],
                                          in_=xT_sb[:, kb].bitcast(f32))
            elif x_blocks in ("k3", "k6"):
                # split along the contraction chunks: keeps 8KB DMA lines
                # and lets the first QKV matmuls start early in the load
                step = 2 if x_blocks == "k3" else 1
                for kb in range(0, KC, step):
                    nc.gpsimd.dma_start(out=xT_sb[:, kb:kb + step],
                                        in_=xTr[:, kb:kb + step])
            elif x_blocks:
                w = N // x_blocks
                for jb in range(x_blocks):
                    jn = slice(jb * w, (jb + 1) * w)
                    nc.gpsimd.dma_start(out=xT_sb[:, :, jn], in_=xTr[:, :, jn])
            else:
                nc.gpsimd.dma_start(out=xT_sb, in_=xTr)
            nc.gpsimd.dma_start(
                out=wk_sb, in_=wk.rearrange("(k p) h d -> p k h d", p=128))
            nc.gpsimd.dma_start(
                out=wq_sb, in_=wq.rearrange("(k p) h d -> p k h d", p=128))

            qk_ps = p1.enter_context(
                tc.tile_pool(name="qk_ps", bufs=3, space="PSUM"))
            v_ps = p1.enter_context(
                tc.tile_pool(name="v_ps", bufs=2, space="PSUM"))

            # V (+ones col): natural [n, h*97+d] = xT[k][:, ntile].T @ Wv[k]
            for t in range(NT):
                ps = v_ps.tile([128, HPC * DA], f32, tag="v")
                for k in range(KC):
                    nc.tensor.matmul(
                        ps, xT_sb[:, k, t * 128:(t + 1) * 128],
                        wv_sb[:, k, :, :].rearrange("p h d -> p (h d)"),
                        start=(k == 0), stop=(k == KC - 1))
                nc.vector.tensor_tensor(
                    out=vA[:, t, :], in0=ps, in1=bvb_sb,
                    op=mybir.AluOpType.add)

            # qT/kT: [d, n] = sum_k Wq[k][:, h-cols].T @ xT[k]
            # head-outer so head h's attention can start while h+1 projects
            for h in range(HPC):
                for (w_sb, b_sb, dst) in ((wk_sb, bk_sb, kT),
                                          (wq_sb, bq_sb, qT)):
                    for j in range(NB):
                        ps = qk_ps.tile([D, 512], f32, tag="qk")
                        for k in range(KC):
                            nc.tensor.matmul(
                                ps, w_sb[:, k, h, :],
                                xT_sb[:, k, j * 512:(j + 1) * 512],
                                start=(k == 0), stop=(k == KC - 1))
                        nc.vector.tensor_scalar(
                            out=dst[0:D, h, j * 512:(j + 1) * 512],
                            in0=ps, scalar1=b_sb[0:D, h:h + 1],
                            scalar2=None, op0=mybir.AluOpType.add)

        # ------- Phases 2-5: attention + normalize + projection -------
        # Query-block-outer so the projection of block J overlaps the
        # attention of block J+1.
        with ExitStack() as p2:
            oT = p2.enter_context(tc.tile_pool(name="p2sb", bufs=1)).tile(
                [128, HPC, N], f32r)             # rows 0:96 o, 96 denom
            expool = p2.enter_context(tc.tile_pool(name="p2ex", bufs=1))
            small2 = p2.enter_context(tc.tile_pool(name="p2small", bufs=3))
            pool5 = p2.enter_context(tc.tile_pool(name="p5sb", bufs=3))
            e_ps = p2.enter_context(
                tc.tile_pool(name="e_ps", bufs=eps_bufs, space="PSUM"))
            o_ps = p2.enter_context(
                tc.tile_pool(name="o_ps", bufs=1, space="PSUM"))
            bc_ps = p2.enter_context(
                tc.tile_pool(name="bc_ps", bufs=1, space="PSUM"))
            p_ps = p2.enter_context(
                tc.tile_pool(name="p_ps", bufs=1, space="PSUM"))
            for j in range(NB):
                jq = slice(j * 512, (j + 1) * 512)
                for h in range(HPC):
                    ex = expool.tile([128, NT, 512], f32r, tag="ex")
                    ops = o_ps.tile([DA, 512], f32, tag="o")
                    gsz = 4 // eps_bufs
                    for grp in range(NT // gsz):
                        eps = e_ps.tile([128, gsz, 512], f32, tag="e")
                        for i in range(gsz):
                            t = grp * gsz + i
                            nc.tensor.matmul(
                                eps[:, i, :], kT[0:D, h, t * 128:(t + 1) * 128],
                                qT[0:D, h, jq], start=True, stop=True)
                        nc.scalar.activation(
                            out=ex[:, grp * gsz:(grp + 1) * gsz, :], in_=eps,
                            func=mybir.ActivationFunctionType.Exp)
                    for t in range(NT):
                        nc.tensor.matmul(
                            ops, vA[:, t, h * DA:(h + 1) * DA], ex[:, t, :],
                            start=(t == 0), stop=(t == NT - 1))
                    nc.vector.tensor_copy(out=oT[0:DA, h, jq], in_=ops)

                    # normalize (h, j): recip of denom row, broadcast, mult
                    # (borrows an energy-psum slot; first bank of the pair)
                    bc = bc_ps.tile([128, 512], f32, tag="bc")
                    with nc.allow_low_precision(
                            reason="f32r is 4-byte; recip is full fp32"):
                        if norm_inplace:
                            nc.vector.reciprocal(out=oT[D:DA, h, jq],
                                                 in_=ops[D:DA, :])
                            nc.tensor.matmul(bc, ones96_sb[D:DA, :],
                                             oT[D:DA, h, jq],
                                             start=True, stop=True,
                                             tile_position=(D - D % 32, 0))
                        else:
                            scat = small2.tile([128, NB], f32r, tag="scat")
                            recip = small2.tile([1, 512], f32r, tag="recip")
                            dma_eng = nc.sync if scat_sync else nc.gpsimd
                            dma_eng.dma_start(out=scat, in_=oT[D:DA, h, jq])
                            nc.vector.reciprocal(out=scat, in_=scat)
                            dma_eng.dma_start(out=recip, in_=scat)
                            nc.tensor.matmul(bc, ones_sb, recip,
                                             start=True, stop=True)
                    nc.vector.tensor_tensor(
                        out=oT[0:D, h, jq], in0=oT[0:D, h, jq],
                        in1=bc[0:D, :], op=mybir.AluOpType.mult)

                # projection of this query block (4 n-tiles of 128)
                for t in range(4 * j, 4 * (j + 1)):
                    tn = slice(t * 128, (t + 1) * 128)
                    ot = pool5.tile([128, E], f32, tag="out")
                    for (lo, hi) in ((0, 512), (512, E)):
                        ps = p_ps.tile([128, hi - lo], f32, tag=f"p{lo}")
                        for h in range(HPC):
                            nc.tensor.matmul(ps, oT[0:D, h, tn],
                                             wp_sb[0:D, h, lo:hi],
                                             start=(h == 0),
                                             stop=(h == HPC - 1))
                        nc.vector.tensor_copy(out=ot[:, lo:hi], in_=ps)
                    nc.sync.dma_start(out=y[tn, :], in_=ot)

    nc.compile()
    return nc


def _get_compiled():
    global _compiled
    if _compiled is None:
        _compiled = _build()
    return _compiled


def make_in_maps(x, Wqkv, bqkv, Wproj, bproj):
    """Host-side sharding: per-core input dict."""
    x = np.asarray(x, dtype=np.float32)
    Wqkv = np.asarray(Wqkv, dtype=np.float32)
    bqkv = np.asarray(bqkv, dtype=np.float32)
    Wproj = np.asarray(Wproj, dtype=np.float32)

    Wr = Wqkv.reshape(E, H, D, 3)
    br = bqkv.reshape(H, D, 3)
    ones = np.ones((1, 128), dtype=np.float32)
    in_maps = []
    for c in range(N_CORES):
        b, g = divmod(c, 2)
        hs = slice(g * HPC, (g + 1) * HPC)
        wv_aug = np.zeros((E, HPC, DA), dtype=np.float32)
        wv_aug[:, :, :D] = Wr[:, hs, :, 2]
        bv_aug = np.zeros((HPC, DA), dtype=np.float32)
        bv_aug[:, :D] = br[hs, :, 2]
        bv_aug[:, D] = 1.0
        in_maps.append({
            "xT": np.ascontiguousarray(x[b].T),
            "wq": np.ascontiguousarray(Wr[:, hs, :, 0]),
            "wk": np.ascontiguousarray(Wr[:, hs, :, 1]),
            "wv": wv_aug,
            "bq": np.ascontiguousarray(br[hs, :, 0]),
            "bk": np.ascontiguousarray(br[hs, :, 1]),
            "bv": bv_aug,
            "wp": np.ascontiguousarray(
                (SCALING * Wproj[g * HPC * D:(g + 1) * HPC * D, :])
                .reshape(HPC, D, E)),
            "ones": ones,
        })
    return in_maps


def combine_outputs(results, bproj):
    bproj = np.asarray(bproj, dtype=np.float32)
    out = np.empty((B, N, E), dtype=np.float32)
    for b in range(B):
        out[b] = results[2 * b]["y"] + results[2 * b + 1]["y"] + bproj
    return out


def kernel(x, Wqkv, bqkv, Wproj, bproj):
    import time
    from concourse.bass_utils import run_bass_kernel_spmd
    nc = _get_compiled()
    in_maps = make_in_maps(x, Wqkv, bqkv, Wproj, bproj)
    last_exc = None
    for attempt in range(3):
        try:
            res = run_bass_kernel_spmd(nc, in_maps,
                                       core_ids=list(range(N_CORES)),
                                       trace=False)
            return combine_outputs(res.results, bproj)
        except Exception as e:  # transient device/terminal failures
            last_exc = e
            time.sleep(10.0)
    raise last_exc
